# revision 33
# baseline (speedup 1.0000x reference)
"""HeteroAttentionLayer (SAGEConv-LSTM aggregator) Bass kernel for 8x TRN2 cores.

Data-parallel over nodes: each core gets 6250 nodes padded to 6272 = 12 blocks
of 512 + 1 mini block of 128. Full x replicated per core as bf16 gather tables
(lo/hi split for int16 dma_gather indices, with dedicated zero rows so the two
gathers merge by one DVE add).

v4 design (on top of v3's rotation pipeline), 741us -> 677us:
- blocks processed in ROTATIONS [5,5,3] sharing a 2-deep PSUM ring; scalar
  engine (sigmoid+tanh floor, ~491us busy) stays saturated; per-block serial
  chain has 4-5x latency slack. Lag-one emission keeps the in-order ACT queue
  from head-blocking on the DVE cell chain.
- PAD shrunk 6656->6272 (12 full blocks + one 128-node mini block) cuts ~6%
  off every engine. The mini block's four 128-col gate groups share one 2KB
  PSUM zero region, so only the first/last matmul of its step carry
  start/stop (a second start would mark earlier groups pending-zero and the
  hh accumulate would overwrite them).
- neigh indices resident per rotation (one fused lo+hi DMA), x node-major
  tile resident (one deferred low-priority DMA); the block's own rows are
  appended to the half-B gather so the node-major->feature-major xT
  transpose comes free with the neighbor gather (no idxx/emit_xtb).
- lo/hi gather merge-adds are emitted at very low scheduler priority so the
  list scheduler drops them into DVE idle slots after the DMA lands instead
  of head-blocking the cell-update stream (prologue merges stay prompt).
- finals split: the fc_self+fc_neigh PSUM matmul + copy-out and the two
  LN+lrelu passes are deferred and dribbled one closure per slot into the
  next rotation's steps, so rotation boundaries never stall ACT/PE on a
  burst of LN chains; the last rotation runs its finals inline (nothing
  follows them).
- banks ordered (f,i,o,g); step 0 skips the f-gate sigmoid+matmul (c0=0) and
  the recurrent matmuls (h0=0); one sigmoid per step covers all live banks
  (tanh gate as 2*sigmoid(2g)-1 via pre-doubled weights, folded in place
  into the sifo tile); layernorm tail is ACT-free (DVE bn_stats + bit-trick
  rsqrt, lrelu on DVE); output stored bf16 and widened on host.
"""
import os
import numpy as np
import ml_dtypes

try:
    from antenv.axon_hooks import get_axon_ntff_profile_hook  # noqa: F401
except Exception:
    os.environ["BASS_NEVER_TRACE"] = "1"

import concourse.bass as bass
import concourse.bacc as bacc
import concourse.tile as tile
from concourse import mybir
from concourse.bass_utils import run_bass_kernel_spmd

N, D, F = 50000, 16, 128
NCORES = 8
SHARD = 6250
PAD = 6272          # 12 * 512 + 128
NBLK = 13
WIDTHS = [512] * 12 + [128]
SPLIT = 32767       # rows 0..32766 -> lo table; 32767.. -> hi table
NLO = 32768         # lo table rows (incl zero row at 32767)
NHI = 17234         # hi table rows (zero row at 0, x[32767:] at 1..17233)
EPS = 1e-5
MAGIC = 0x5F3759DF  # fast inverse-sqrt seed
ROTS = [list(range(0, 4)), list(range(4, 9)), list(range(9, 13))]
IDXC = 288          # idx cols per half (16 rows per col); half-B needs 288

fp32 = mybir.dt.float32
bf16 = mybir.dt.bfloat16
i16 = mybir.dt.int16
i32 = mybir.dt.int32
AF = mybir.ActivationFunctionType
OP = mybir.AluOpType

_CACHE = {}
EMIT_LOG = []


def _geom(b):
    w = WIDTHS[b]
    ha = 8 * w            # positions in half A (steps 0-7)
    hb = 8 * w + w        # half B (steps 8-15) + the block's own rows (xT)
    return w, ha, hb


def _build(use_bias_g, use_bias_o, ln1_aff, ln3_aff):
    nc = bacc.Bacc()
    EMIT_LOG.clear()

    def mark(label):
        EMIT_LOG.append((int(nc.get_next_instruction_name()[2:]), label))

    xlo = nc.dram_tensor("xlo", [NLO, F], bf16, kind="ExternalInput")
    xhi = nc.dram_tensor("xhi", [NHI, F], bf16, kind="ExternalInput")
    xsh = nc.dram_tensor("xsh", [PAD, F], bf16, kind="ExternalInput")
    idxt = nc.dram_tensor("idxt", [NBLK, 2, 128, 2 * IDXC], i16, kind="ExternalInput")
    wih = nc.dram_tensor("wih", [F, 4 * F], bf16, kind="ExternalInput")   # cols: f,i,o,2g
    whh = nc.dram_tensor("whh", [F, 4 * F], bf16, kind="ExternalInput")
    wself = nc.dram_tensor("wself", [F, F], bf16, kind="ExternalInput")
    wneigh = nc.dram_tensor("wneigh", [F, F], bf16, kind="ExternalInput")
    bg = nc.dram_tensor("bg", [F, 4], fp32, kind="ExternalInput")          # b_ih+b_hh per gate
    bo = nc.dram_tensor("bo", [128, F], fp32, kind="ExternalInput")        # b_self+b_neigh bcast
    g1t = nc.dram_tensor("g1t", [128, F], fp32, kind="ExternalInput")
    b1t = nc.dram_tensor("b1t", [128, F], fp32, kind="ExternalInput")
    g3t = nc.dram_tensor("g3t", [128, F], fp32, kind="ExternalInput")
    b3t = nc.dram_tensor("b3t", [128, F], fp32, kind="ExternalInput")
    out = nc.dram_tensor("out", [PAD, F], bf16, kind="ExternalOutput")

    xsh_v = xsh.rearrange("(q p) f -> p q f", p=128)     # q = 49 row-chunks
    out_v = out.rearrange("(q p) f -> p q f", p=128)
    idxt_v = idxt.rearrange("b h p c -> p b h c")

    with tile.TileContext(nc) as tc:
        with (
            tc.tile_pool(name="consts", bufs=1) as consts,
            tc.tile_pool(name="idxp", bufs=2) as idxp,
            tc.tile_pool(name="mt", bufs=1) as pmt,
            tc.tile_pool(name="state", bufs=3) as pst,
            tc.tile_pool(name="work", bufs=3) as pwk,
            tc.tile_pool(name="fin", bufs=2) as pfin,
            tc.tile_pool(name="psum", bufs=2, space="PSUM") as pps,
        ):
            def load_rot_idx(rot, split=False):
                # rotation 0 splits per block so block 0's gathers can start
                # after an 0.8us transfer; later rotations use one DMA
                it = idxp.tile([128, 5, 2, 2 * IDXC], i16, tag="it", name="it")
                nb = len(rot)
                if split:
                    for k, b in enumerate(rot):
                        nc.sync.dma_start(out=it[:, k:k + 1],
                                          in_=idxt_v[:, b:b + 1])
                else:
                    nc.sync.dma_start(out=it[:, 0:nb],
                                      in_=idxt_v[:, rot[0]:rot[0] + nb])
                return it[:, :, :, 0:IDXC], it[:, :, :, IDXC:2 * IDXC]

            ia0, ib0 = load_rot_idx(ROTS[0])
            w_ih = consts.tile([F, 4 * F], bf16)
            nc.sync.dma_start(out=w_ih[:], in_=wih[:])
            w_hh = consts.tile([F, 4 * F], bf16)
            nc.sync.dma_start(out=w_hh[:], in_=whh[:])
            w_s = consts.tile([F, F], bf16)
            nc.sync.dma_start(out=w_s[:], in_=wself[:])
            w_n = consts.tile([F, F], bf16)
            nc.sync.dma_start(out=w_n[:], in_=wneigh[:])
            magic = consts.tile([128, 8], i32)
            nc.vector.memset(magic[:], float(MAGIC))
            bg_sb = consts.tile([F, 4], fp32)
            if use_bias_g:
                nc.sync.dma_start(out=bg_sb[:], in_=bg[:])
            bo_sb = consts.tile([128, F], fp32)
            if use_bias_o:
                nc.sync.dma_start(out=bo_sb[:], in_=bo[:])
            if ln1_aff:
                g1_sb = consts.tile([128, F], fp32)
                b1_sb = consts.tile([128, F], fp32)
                nc.sync.dma_start(out=g1_sb[:], in_=g1t[:])
                nc.sync.dma_start(out=b1_sb[:], in_=b1t[:])
            if ln3_aff:
                g3_sb = consts.tile([128, F], fp32)
                b3_sb = consts.tile([128, F], fp32)
                nc.sync.dma_start(out=g3_sb[:], in_=g3t[:])
                nc.sync.dma_start(out=b3_sb[:], in_=b3t[:])
            # node-major x for the residual: one DMA, resident all kernel
            # (the dma_start is deferred into the loop so its 6272
            # descriptors don't block the prologue gathers)
            xb_all = consts.tile([128, PAD // 128, F], bf16)

            def rsqrt_dve(y, v, nvals):
                vi = v.bitcast(i32)
                sh = pfin.tile([128, 8], i32, tag="rs_sh")
                nc.vector.tensor_single_scalar(
                    out=sh[:, 0:nvals], in_=vi, scalar=1, op=OP.logical_shift_right)
                yi = y.bitcast(i32)
                nc.vector.scalar_tensor_tensor(
                    out=yi, in0=magic[:, 0:nvals], scalar=0.0, in1=sh[:, 0:nvals],
                    op0=OP.bypass, op1=OP.subtract)
                a = pfin.tile([128, 8], fp32, tag="rs_a")
                nc.vector.tensor_mul(out=a[:, 0:nvals], in0=y, in1=y)
                nc.vector.tensor_mul(out=a[:, 0:nvals], in0=a[:, 0:nvals], in1=v)
                nc.vector.tensor_scalar(
                    out=a[:, 0:nvals], in0=a[:, 0:nvals], scalar1=-0.5, scalar2=1.5,
                    op0=OP.mult, op1=OP.add)
            # 1 Newton step
                nc.vector.tensor_mul(out=y, in0=y, in1=a[:, 0:nvals])

            def emit_gather_pair(S, half, q0, q1):
                # gather positions [q0,q1) of the given half of block S: one
                # lo-table + one hi-table gather; the merging DVE add is
                # emitted later (emit_merge) once the DMA has surely landed,
                # so it never head-blocks the in-order DVE queue
                pn = q1 - q0
                mt = S["mt0"] if half == 0 else S["mt1"]
                ia, ib, s = S["ia"], S["ib"], S["slot"]
                nc.gpsimd.dma_gather(
                    out_ap=mt[:, 0:1, q0:q1], in_ap=xlo[:],
                    idxs_ap=ia[:, s, half, q0 // 16:q1 // 16],
                    num_idxs=pn, num_idxs_reg=pn, elem_size=F, transpose=True,
                    single_packet=False)
                sc = pmt.tile([128, 1, 2560], bf16, tag="sc", bufs=4, name="sc")
                nc.gpsimd.dma_gather(
                    out_ap=sc[:, :, 0:pn], in_ap=xhi[:],
                    idxs_ap=ib[:, s, half, q0 // 16:q1 // 16],
                    num_idxs=pn, num_idxs_reg=pn, elem_size=F, transpose=True,
                    single_packet=False)
                return (S, half, q0, q1, sc)

            def emit_merge(t, defer=True):
                # deferred merges get low priority: the list scheduler slots
                # them into DVE idle time once the gather DMA has landed,
                # instead of head-blocking the cell-update stream
                S, half, q0, q1, sc = t
                pn = q1 - q0
                mt = S["mt0"] if half == 0 else S["mt1"]
                with tc.high_priority(offset=-1000000 if defer else 0):
                    nc.vector.tensor_add(out=mt[:, 0, q0:q1],
                                         in0=mt[:, 0, q0:q1],
                                         in1=sc[:, 0, 0:pn])

            def emit_gather_part(S, half, q0, q1, defer=True):
                emit_merge(emit_gather_pair(S, half, q0, q1), defer=defer)

            def alloc_mt(S):
                s = S["slot"]
                S["mt0"] = pmt.tile([128, 1, 4096], bf16, tag=f"mt0{s}", bufs=1,
                                    name=f"mt0{s}")
                S["mt1"] = pmt.tile([128, 1, 4608], bf16, tag=f"mt1{s}", bufs=1,
                                    name=f"mt1{s}")

            def emit_step_mm_sigma(S, d):
                w = S["w"]
                mt = S["mt0"] if d < 8 else S["mt1"]
                col = (d % 8) * w
                g = pps.tile([128, 2048], fp32, tag="g")
                S["g"] = g
                g0 = 1 if d == 0 else 0   # f-gate dead at d==0 (c0 = 0)
                # PSUM accumulation groups operate on 2KB zero regions: for
                # w=512 each gate bank owns a region (per-bank start/stop);
                # for w=128 all four banks share one region (single group).
                for gi in range(g0, 4):
                    nc.tensor.matmul(
                        out=g[:, gi * w:(gi + 1) * w],
                        lhsT=w_ih[:, gi * 128:(gi + 1) * 128],
                        rhs=mt[:, 0, col:col + w],
                        start=(w == 512 or gi == g0),
                        stop=(d == 0 and (w == 512 or gi == 3)))
                if d > 0:
                    for gi in range(4):
                        nc.tensor.matmul(
                            out=g[:, gi * w:(gi + 1) * w],
                            lhsT=w_hh[:, gi * 128:(gi + 1) * 128],
                            rhs=S["h"][:, 0:w], start=False,
                            stop=(w == 512 or gi == 3))
                sifo = pwk.tile([128, 2048], bf16, tag="sifo", bufs=4)
                if use_bias_g:
                    for k in range(g0, 4):
                        nc.scalar.activation(
                            out=sifo[:, k * w:(k + 1) * w],
                            in_=g[:, k * w:(k + 1) * w],
                            func=AF.Sigmoid, bias=bg_sb[:, k:k + 1])
                else:
                    nc.scalar.activation(out=sifo[:, g0 * w:4 * w],
                                         in_=g[:, g0 * w:4 * w], func=AF.Sigmoid)
                S["sifo"] = sifo

            def emit_step_rest(S, d):
                # DVE cell update + tanh(c) + h; emitted one slot late so the
                # in-order ACT queue never head-blocks on the DVE chain
                w, s, sifo = S["w"], S["slot"], S["sifo"]
                c = pst.tile([128, 512], bf16, tag=f"c{s}")
                if d == 0:
                    nc.vector.tensor_scalar(
                        out=c[:, 0:w], in0=sifo[:, 3 * w:4 * w], scalar1=2.0,
                        scalar2=1.0, op0=OP.mult, op1=OP.subtract)
                    nc.vector.tensor_mul(out=c[:, 0:w], in0=c[:, 0:w],
                                         in1=sifo[:, w:2 * w])
                else:
                    nc.vector.tensor_scalar(
                        out=sifo[:, 3 * w:4 * w], in0=sifo[:, 3 * w:4 * w],
                        scalar1=2.0, scalar2=1.0, op0=OP.mult, op1=OP.subtract)
                    c2 = pwk.tile([128, 512], bf16, tag="c2", bufs=3)
                    nc.vector.tensor_mul(out=c2[:, 0:w], in0=sifo[:, 0:w],
                                         in1=S["c"][:, 0:w])
                    nc.vector.tensor_mul(out=c[:, 0:w], in0=sifo[:, 3 * w:4 * w],
                                         in1=sifo[:, w:2 * w])
                    nc.vector.tensor_add(out=c[:, 0:w], in0=c[:, 0:w],
                                         in1=c2[:, 0:w])
                S["c"] = c
                tc_ = pwk.tile([128, 512], bf16, tag="tc", bufs=3)
                nc.scalar.activation(out=tc_[:, 0:w], in_=c[:, 0:w], func=AF.Tanh)
                h = pst.tile([128, 512], bf16, tag=f"h{s}")
                nc.vector.tensor_mul(out=h[:, 0:w], in0=sifo[:, 2 * w:3 * w],
                                     in1=tc_[:, 0:w])
                S["h"] = h

            def emit_final_psum(S):
                # rst directly in node-major: rst[node,f'] = xT.T @ W_selfT
                # + hT.T @ W_neighT; copied out of PSUM immediately so the
                # gates ring never backs up at a rotation boundary.
                w, ns = S["w"], S["w"] // 128
                xT = S["mt1"][:, 0, 8 * w:9 * w]
                rp = pps.tile([128, 2048], fp32, tag="g")
                v = rp[:, 0:512].rearrange("p (s f) -> p s f", s=4)
                for s4 in range(ns):
                    nc.tensor.matmul(
                        out=v[:, s4, :],
                        lhsT=xT[:, s4 * 128:(s4 + 1) * 128],
                        rhs=w_s[:], start=True, stop=False)
                    nc.tensor.matmul(
                        out=v[:, s4, :],
                        lhsT=S["h"][:, s4 * 128:(s4 + 1) * 128],
                        rhs=w_n[:], start=False, stop=True)
                rst = pfin.tile([128, 4, F], bf16, tag="rst", bufs=6)
                nc.vector.tensor_copy(out=rst[:, 0:ns], in_=v[:, 0:ns])
                if use_bias_o:
                    for s4 in range(ns):
                        nc.vector.tensor_add(out=rst[:, s4, :], in0=rst[:, s4, :],
                                             in1=bo_sb[:])
                S["rst"] = rst

            def ln_stats(S, key, srckey):
                ns = S["w"] // 128
                src = S[srckey]
                st = pfin.tile([128, 4, 6], fp32, tag="lnst", bufs=3, name="st")
                mv = pfin.tile([128, 4, 2], fp32, tag="lnmv", bufs=3, name="mv")
                for s in range(ns):
                    nc.vector.bn_stats(out=st[:, s, :], in_=src[:, s, :])
                    nc.vector.bn_aggr(out=mv[:, s, :], in_=st[:, s, :])
                S[key] = mv

            def ln_apply(S, dst, src, mv, aff, gsb, bsb):
                ns = S["w"] // 128
                ve = pfin.tile([128, 4], fp32, tag="lnve")
                nc.vector.tensor_scalar(
                    out=ve[:, 0:ns], in0=mv[:, 0:ns, 1], scalar1=EPS, scalar2=None,
                    op0=OP.add, op1=OP.bypass)
                rstd = pfin.tile([128, 4], fp32, tag="lnrstd")
                rsqrt_dve(rstd[:, 0:ns], ve[:, 0:ns], ns)
                for s in range(ns):
                    nc.vector.tensor_scalar(
                        out=dst[:, s, :], in0=src[:, s, :],
                        scalar1=mv[:, s, 0:1], scalar2=rstd[:, s:s + 1],
                        op0=OP.subtract, op1=OP.mult)
                    if aff:
                        nc.vector.tensor_mul(out=dst[:, s, :], in0=dst[:, s, :], in1=gsb[:])
                        nc.vector.tensor_add(out=dst[:, s, :], in0=dst[:, s, :], in1=bsb[:])
                dstf = dst[:, 0:ns].rearrange("p s f -> p (s f)")
                nc.vector.scalar_tensor_tensor(
                    out=dstf, in0=dstf, scalar=0.01, in1=dstf,
                    op0=OP.mult, op1=OP.max)

            def fin_ln1s(S):
                ln_stats(S, "mv1", "rst")

            def fin_ln1a(S):
                rst, ns = S["rst"], S["w"] // 128
                ln_apply(S, rst, rst, S["mv1"],
                         ln1_aff, g1_sb if ln1_aff else None,
                         b1_sb if ln1_aff else None)
                h2 = pfin.tile([128, 4, F], bf16, tag="h2", bufs=3)
                q = 4 * S["b"]
                nc.vector.tensor_add(out=h2[:, 0:ns], in0=rst[:, 0:ns],
                                     in1=xb_all[:, q:q + ns, :])
                S["h2"] = h2

            def fin_ln3s(S):
                ln_stats(S, "mv3", "h2")

            def fin_ln3a(S):
                b, ns = S["b"], S["w"] // 128
                ot = pfin.tile([128, 4, F], bf16, tag="ot", bufs=3)
                ln_apply(S, ot, S["h2"], S["mv3"],
                         ln3_aff, g3_sb if ln3_aff else None,
                         b3_sb if ln3_aff else None)
                nc.sync.dma_start(out=out_v[:, 4 * b:4 * b + ns, :], in_=ot[:, 0:ns])

            FIN_CHAIN = (fin_ln1s, fin_ln1a, fin_ln3s, fin_ln3a)

            # ---------------- schedule ----------------
            states = {}

            def init_states(rot, ia, ib):
                for s, b in enumerate(rot):
                    S = states[b] = {"b": b, "slot": s, "w": WIDTHS[b],
                                     "ia": ia, "ib": ib}
                    alloc_mt(S)
                return [states[b] for b in rot]

            pending = [None]
            pending_ln = []
            pending_fin = []
            inline_fin = [False]    # last rotation: finals inline (no successor)

            def finish_block(S):
                if inline_fin[0]:
                    emit_final_psum(S)
                    for fn in FIN_CHAIN:
                        fn(S)
                else:
                    pending_fin.append(S)

            def emit_mm_sigma_lagged(S, d):
                emit_step_mm_sigma(S, d)
                prev, pending[0] = pending[0], (S, d)
                if prev is not None:
                    emit_step_rest(prev[0], prev[1])
                    if prev[1] == D - 1:
                        finish_block(prev[0])

            def flush_pending():
                prev, pending[0] = pending[0], None
                if prev is not None:
                    emit_step_rest(prev[0], prev[1])
                    if prev[1] == D - 1:
                        finish_block(prev[0])

            def dribble_fin():
                # one unit of deferred final work per slot: first the PSUM
                # matmul+copy of a finished block, then its LN closures
                if pending_fin:
                    S = pending_fin.pop(0)
                    emit_final_psum(S)
                    pending_ln.extend((fn, S) for fn in FIN_CHAIN)
                elif pending_ln:
                    fn, S = pending_ln.pop(0)
                    fn(S)

            # prologue: rotation 0 half-A parts in need-time order (the
            # idx DMA was emitted first, ahead of the weight loads)
            rs = init_states(ROTS[0], ia0, ib0)
            parts = []
            for S in rs:
                w = S["w"]
                bnds = [0, w, 3 * w, 8 * w]
                for p in range(len(bnds) - 1):
                    key = (bnds[p] // w) * 5 + S["slot"]
                    parts.append((key, S["slot"], S, 0, bnds[p], bnds[p + 1]))
            parts.sort(key=lambda t: (t[0], t[1]))
            for _, _, S, half, q0, q1 in parts:
                emit_gather_part(S, half, q0, q1, defer=False)

            for r, rot in enumerate(ROTS):
                nxt = ROTS[r + 1] if r + 1 < len(ROTS) else None
                inline_fin[0] = nxt is None
                cur = [states[b] for b in rot]
                for d in range(1 if r > 0 else 0, D):
                    # gather parts to spread one-per-slot over this step;
                    # their merge-adds run 3 slots later (the DMA has landed
                    # by then, so the DVE queue never parks on a gather)
                    gq = []
                    if d in (0, 1) and r == 0:
                        for S in cur:
                            hb = 9 * S["w"]
                            mid = (hb // 2) // 128 * 128
                            q0, q1 = (0, mid) if d == 0 else (mid, hb)
                            gq.append((S, 1, q0, q1))
                    if d == 6 and nxt is not None:
                        with tc.high_priority(offset=-1000000):
                            nia, nib = load_rot_idx(nxt)
                        nxt_s = init_states(nxt, nia, nib)
                    if d in (7, 8) and nxt is not None:
                        for S2 in nxt_s:
                            ha = 8 * S2["w"]
                            q0, q1 = (0, ha // 2) if d == 7 else (ha // 2, ha)
                            gq.append((S2, 0, q0, q1))
                    if d in (10, 11) and nxt is not None:
                        for S2 in nxt_s:
                            hb = 9 * S2["w"]
                            mid = (hb // 2) // 128 * 128
                            q0, q1 = (0, mid) if d == 10 else (mid, hb)
                            gq.append((S2, 1, q0, q1))
                    for S in cur:
                        mark(f"r{r}d{d}s{S['slot']}")
                        emit_mm_sigma_lagged(S, d)
                        if gq:
                            emit_gather_part(*gq.pop(0))
                        if d >= 1:
                            dribble_fin()
                    for t in gq:
                        emit_gather_part(*t)
                    if d == 2 and r == 0:
                        with tc.high_priority(offset=-1000000):
                            nc.sync.dma_start(out=xb_all[:], in_=xsh_v[:])
                # boundary: next rotation's step 0 (no h dependency) goes
                # ahead of this rotation's lagged final work
                if nxt is not None:
                    for S2 in nxt_s:
                        mark(f"r{r+1}d0s{S2['slot']}-ovl")
                        emit_mm_sigma_lagged(S2, 0)
                flush_pending()
            while pending_fin or pending_ln:
                dribble_fin()

    nc.compile()
    return nc


def _wrap16(vals):
    # vals [..., n] -> [..., 128, n//16] int16 (16-wrap, x8 replicate)
    lead = vals.shape[:-1]
    n = vals.shape[-1]
    w = vals.reshape(*lead, n // 16, 16)
    w = np.moveaxis(w, -1, -2)                     # [..., 16, n/16]
    w = np.broadcast_to(w[..., None, :, :], (*lead, 8, 16, n // 16))
    return np.ascontiguousarray(w.reshape(*lead, 128, n // 16)).astype(np.int16)


def kernel(x, neigh_idx, W_self, b_self, W_neigh, b_neigh,
           W_ih, W_hh, b_ih, b_hh, g1, bt1, g3, bt3):
    bf = ml_dtypes.bfloat16
    x = np.asarray(x, np.float32)
    neigh_idx = np.asarray(neigh_idx, np.int32)

    xlo = np.zeros((NLO, F), bf)
    xlo[:SPLIT] = x[:SPLIT]                     # row 32767 stays zero
    xhi = np.zeros((NHI, F), bf)
    xhi[1:1 + (N - SPLIT)] = x[SPLIT:]          # row 0 stays zero

    # gate order in reference: i, f, g, o ; we use banks (f, i, o, g)
    perm = np.concatenate([np.arange(128, 256), np.arange(0, 128),
                           np.arange(384, 512), np.arange(256, 384)])
    gscale = np.ones((1, 512), np.float32)
    gscale[:, 384:512] = 2.0                    # tanh gate via 2*sigmoid(2g)-1
    W_ihT = np.ascontiguousarray(
        (np.asarray(W_ih, np.float32).T[:, perm] * gscale)).astype(bf)
    W_hhT = np.ascontiguousarray(
        (np.asarray(W_hh, np.float32).T[:, perm] * gscale)).astype(bf)
    bgv = (np.asarray(b_ih, np.float32) + np.asarray(b_hh, np.float32))[perm]
    bgv = bgv * gscale[0]
    bg2 = np.ascontiguousarray(bgv.reshape(4, F).T)                        # [F, 4]
    W_selfT = np.ascontiguousarray(np.asarray(W_self, np.float32).T).astype(bf)
    W_neighT = np.ascontiguousarray(np.asarray(W_neigh, np.float32).T).astype(bf)
    bov = np.ascontiguousarray(np.broadcast_to(
        np.asarray(b_self, np.float32) + np.asarray(b_neigh, np.float32), (128, F)))

    g1 = np.asarray(g1, np.float32); bt1 = np.asarray(bt1, np.float32)
    g3 = np.asarray(g3, np.float32); bt3 = np.asarray(bt3, np.float32)
    use_bias_g = bool(np.any(bgv != 0))
    use_bias_o = bool(np.any(bov != 0))
    ln1_aff = bool(np.any(g1 != 1) or np.any(bt1 != 0))
    ln3_aff = bool(np.any(g3 != 1) or np.any(bt3 != 0))
    g1t = np.ascontiguousarray(np.broadcast_to(g1, (128, F)))
    b1t = np.ascontiguousarray(np.broadcast_to(bt1, (128, F)))
    g3t = np.ascontiguousarray(np.broadcast_to(g3, (128, F)))
    b3t = np.ascontiguousarray(np.broadcast_to(bt3, (128, F)))

    key = (use_bias_g, use_bias_o, ln1_aff, ln3_aff)
    if key not in _CACHE:
        _CACHE[key] = _build(*key)
    nc = _CACHE[key]

    starts = np.cumsum([0] + WIDTHS)

    in_maps = []
    for core in range(NCORES):
        lo_r, hi_r = core * SHARD, (core + 1) * SHARD
        ni_pad = np.zeros((PAD, D), np.int64)
        ni_pad[:SHARD] = neigh_idx[lo_r:hi_r]
        xs_pad = np.zeros((PAD, F), bf)
        xs_pad[:SHARD] = x[lo_r:hi_r]
        # own global row id per padded node (pad rows -> lo zero row)
        own = np.full(PAD, SPLIT, np.int64)
        own[:SHARD] = np.arange(lo_r, hi_r)

        ia_list = np.full((NBLK, 2, 16 * IDXC), SPLIT, np.int64)
        ib_list = np.zeros((NBLK, 2, 16 * IDXC), np.int64)
        for b in range(NBLK):
            w, ha, hb = _geom(b)
            j = np.arange(D * w)
            node = j % w
            dd = j // w
            rows = ni_pad[starts[b] + node, dd]          # [D*w]
            rows = np.concatenate([rows, own[starts[b]:starts[b] + w]])
            lo_mask = rows < SPLIT
            ia = np.where(lo_mask, rows, SPLIT)
            ib = np.where(lo_mask, 0, rows - SPLIT + 1)
            ia_list[b, 0, :ha] = ia[:ha]
            ia_list[b, 1, :hb] = ia[ha:]
            ib_list[b, 0, :ha] = ib[:ha]
            ib_list[b, 1, :hb] = ib[ha:]

        idxt = np.concatenate([_wrap16(ia_list), _wrap16(ib_list)], axis=-1)
        in_maps.append(dict(
            xlo=xlo, xhi=xhi, xsh=xs_pad, idxt=idxt,
            wih=W_ihT, whh=W_hhT, wself=W_selfT, wneigh=W_neighT,
            bg=bg2, bo=bov, g1t=g1t, b1t=b1t, g3t=g3t, b3t=b3t,
        ))

    res = run_bass_kernel_spmd(nc, in_maps, core_ids=list(range(NCORES)))
    kernel.last_results = res
    out = np.concatenate([res.results[c]["out"][:SHARD] for c in range(NCORES)], 0)
    return out.astype(np.float32)


# revision 34
# speedup vs baseline: 1.0162x; 1.0162x over previous
"""HeteroAttentionLayer (SAGEConv-LSTM aggregator) Bass kernel for 8x TRN2 cores.

Data-parallel over nodes: each core gets 6250 nodes padded to 6272 = 12 blocks
of 512 + 1 mini block of 128. Full x replicated per core as bf16 gather tables
(lo/hi split for int16 dma_gather indices, with dedicated zero rows so the two
gathers merge by one DVE add).

v4 design (on top of v3's rotation pipeline), 741us -> 677us:
- blocks processed in ROTATIONS [5,5,3] sharing a 2-deep PSUM ring; scalar
  engine (sigmoid+tanh floor, ~491us busy) stays saturated; per-block serial
  chain has 4-5x latency slack. Lag-one emission keeps the in-order ACT queue
  from head-blocking on the DVE cell chain.
- PAD shrunk 6656->6272 (12 full blocks + one 128-node mini block) cuts ~6%
  off every engine. The mini block's four 128-col gate groups share one 2KB
  PSUM zero region, so only the first/last matmul of its step carry
  start/stop (a second start would mark earlier groups pending-zero and the
  hh accumulate would overwrite them).
- neigh indices resident per rotation (one fused lo+hi DMA), x node-major
  tile resident (one deferred low-priority DMA); the block's own rows are
  appended to the half-B gather so the node-major->feature-major xT
  transpose comes free with the neighbor gather (no idxx/emit_xtb).
- lo/hi gather merge-adds are emitted at very low scheduler priority so the
  list scheduler drops them into DVE idle slots after the DMA lands instead
  of head-blocking the cell-update stream (prologue merges stay prompt).
- finals split: the fc_self+fc_neigh PSUM matmul + copy-out and the two
  LN+lrelu passes are deferred and dribbled one closure per slot into the
  next rotation's steps, so rotation boundaries never stall ACT/PE on a
  burst of LN chains; the last rotation runs its finals inline (nothing
  follows them).
- banks ordered (f,i,o,g); step 0 skips the f-gate sigmoid+matmul (c0=0) and
  the recurrent matmuls (h0=0); one sigmoid per step covers all live banks
  (tanh gate as 2*sigmoid(2g)-1 via pre-doubled weights, folded in place
  into the sifo tile); layernorm tail is ACT-free (DVE bn_stats + bit-trick
  rsqrt, lrelu on DVE); output stored bf16 and widened on host.
"""
import os
import numpy as np
import ml_dtypes

try:
    from antenv.axon_hooks import get_axon_ntff_profile_hook  # noqa: F401
except Exception:
    os.environ["BASS_NEVER_TRACE"] = "1"

import concourse.bass as bass
import concourse.bacc as bacc
import concourse.tile as tile
from concourse import mybir
from concourse.bass_utils import run_bass_kernel_spmd

N, D, F = 50000, 16, 128
NCORES = 8
SHARD = 6250
PAD = 6272          # 12 * 512 + 128
NBLK = 13
WIDTHS = [512] * 12 + [128]
SPLIT = 32767       # rows 0..32766 -> lo table; 32767.. -> hi table
NLO = 32768         # lo table rows (incl zero row at 32767)
NHI = 17234         # hi table rows (zero row at 0, x[32767:] at 1..17233)
EPS = 1e-5
MAGIC = 0x5F3759DF  # fast inverse-sqrt seed
ROTS = [list(range(0, 5)), list(range(5, 10)), list(range(10, 13))]
IDXC = 288          # idx cols per half (16 rows per col); half-B needs 288

fp32 = mybir.dt.float32
bf16 = mybir.dt.bfloat16
i16 = mybir.dt.int16
i32 = mybir.dt.int32
AF = mybir.ActivationFunctionType
OP = mybir.AluOpType

_CACHE = {}
EMIT_LOG = []


def _geom(b):
    w = WIDTHS[b]
    ha = 8 * w            # positions in half A (steps 0-7)
    hb = 8 * w + w        # half B (steps 8-15) + the block's own rows (xT)
    return w, ha, hb


def _build(use_bias_g, use_bias_o, ln1_aff, ln3_aff):
    nc = bacc.Bacc()
    EMIT_LOG.clear()

    def mark(label):
        EMIT_LOG.append((int(nc.get_next_instruction_name()[2:]), label))

    xlo = nc.dram_tensor("xlo", [NLO, F], bf16, kind="ExternalInput")
    xhi = nc.dram_tensor("xhi", [NHI, F], bf16, kind="ExternalInput")
    xsh = nc.dram_tensor("xsh", [PAD, F], bf16, kind="ExternalInput")
    idxt = nc.dram_tensor("idxt", [NBLK, 2, 128, 2 * IDXC], i16, kind="ExternalInput")
    wih = nc.dram_tensor("wih", [F, 4 * F], bf16, kind="ExternalInput")   # cols: f,i,o,2g
    whh = nc.dram_tensor("whh", [F, 4 * F], bf16, kind="ExternalInput")
    wself = nc.dram_tensor("wself", [F, F], bf16, kind="ExternalInput")
    wneigh = nc.dram_tensor("wneigh", [F, F], bf16, kind="ExternalInput")
    bg = nc.dram_tensor("bg", [F, 4], fp32, kind="ExternalInput")          # b_ih+b_hh per gate
    bo = nc.dram_tensor("bo", [128, F], fp32, kind="ExternalInput")        # b_self+b_neigh bcast
    g1t = nc.dram_tensor("g1t", [128, F], fp32, kind="ExternalInput")
    b1t = nc.dram_tensor("b1t", [128, F], fp32, kind="ExternalInput")
    g3t = nc.dram_tensor("g3t", [128, F], fp32, kind="ExternalInput")
    b3t = nc.dram_tensor("b3t", [128, F], fp32, kind="ExternalInput")
    out = nc.dram_tensor("out", [PAD, F], bf16, kind="ExternalOutput")

    xsh_v = xsh.rearrange("(q p) f -> p q f", p=128)     # q = 49 row-chunks
    out_v = out.rearrange("(q p) f -> p q f", p=128)
    idxt_v = idxt.rearrange("b h p c -> p b h c")

    with tile.TileContext(nc) as tc:
        with (
            tc.tile_pool(name="consts", bufs=1) as consts,
            tc.tile_pool(name="idxp", bufs=2) as idxp,
            tc.tile_pool(name="mt", bufs=1) as pmt,
            tc.tile_pool(name="state", bufs=3) as pst,
            tc.tile_pool(name="work", bufs=3) as pwk,
            tc.tile_pool(name="fin", bufs=2) as pfin,
            tc.tile_pool(name="psum", bufs=2, space="PSUM") as pps,
        ):
            def load_rot_idx(rot, split=False):
                # rotation 0 splits per block so block 0's gathers can start
                # after an 0.8us transfer; later rotations use one DMA
                it = idxp.tile([128, 5, 2, 2 * IDXC], i16, tag="it", name="it")
                nb = len(rot)
                if split:
                    for k, b in enumerate(rot):
                        nc.sync.dma_start(out=it[:, k:k + 1],
                                          in_=idxt_v[:, b:b + 1])
                else:
                    nc.sync.dma_start(out=it[:, 0:nb],
                                      in_=idxt_v[:, rot[0]:rot[0] + nb])
                return it[:, :, :, 0:IDXC], it[:, :, :, IDXC:2 * IDXC]

            ia0, ib0 = load_rot_idx(ROTS[0])
            w_ih = consts.tile([F, 4 * F], bf16)
            nc.sync.dma_start(out=w_ih[:], in_=wih[:])
            w_hh = consts.tile([F, 4 * F], bf16)
            nc.sync.dma_start(out=w_hh[:], in_=whh[:])
            w_s = consts.tile([F, F], bf16)
            nc.sync.dma_start(out=w_s[:], in_=wself[:])
            w_n = consts.tile([F, F], bf16)
            nc.sync.dma_start(out=w_n[:], in_=wneigh[:])
            magic = consts.tile([128, 8], i32)
            nc.vector.memset(magic[:], float(MAGIC))
            bg_sb = consts.tile([F, 4], fp32)
            if use_bias_g:
                nc.sync.dma_start(out=bg_sb[:], in_=bg[:])
            bo_sb = consts.tile([128, F], fp32)
            if use_bias_o:
                nc.sync.dma_start(out=bo_sb[:], in_=bo[:])
            if ln1_aff:
                g1_sb = consts.tile([128, F], fp32)
                b1_sb = consts.tile([128, F], fp32)
                nc.sync.dma_start(out=g1_sb[:], in_=g1t[:])
                nc.sync.dma_start(out=b1_sb[:], in_=b1t[:])
            if ln3_aff:
                g3_sb = consts.tile([128, F], fp32)
                b3_sb = consts.tile([128, F], fp32)
                nc.sync.dma_start(out=g3_sb[:], in_=g3t[:])
                nc.sync.dma_start(out=b3_sb[:], in_=b3t[:])
            # node-major x for the residual: one DMA, resident all kernel
            # (the dma_start is deferred into the loop so its 6272
            # descriptors don't block the prologue gathers)
            xb_all = consts.tile([128, PAD // 128, F], bf16)

            def rsqrt_dve(y, v, nvals):
                vi = v.bitcast(i32)
                sh = pfin.tile([128, 8], i32, tag="rs_sh")
                nc.vector.tensor_single_scalar(
                    out=sh[:, 0:nvals], in_=vi, scalar=1, op=OP.logical_shift_right)
                yi = y.bitcast(i32)
                nc.vector.scalar_tensor_tensor(
                    out=yi, in0=magic[:, 0:nvals], scalar=0.0, in1=sh[:, 0:nvals],
                    op0=OP.bypass, op1=OP.subtract)
                a = pfin.tile([128, 8], fp32, tag="rs_a")
                nc.vector.tensor_mul(out=a[:, 0:nvals], in0=y, in1=y)
                nc.vector.tensor_mul(out=a[:, 0:nvals], in0=a[:, 0:nvals], in1=v)
                nc.vector.tensor_scalar(
                    out=a[:, 0:nvals], in0=a[:, 0:nvals], scalar1=-0.5, scalar2=1.5,
                    op0=OP.mult, op1=OP.add)
            # 1 Newton step
                nc.vector.tensor_mul(out=y, in0=y, in1=a[:, 0:nvals])

            def emit_gather_pair(S, half, q0, q1):
                # gather positions [q0,q1) of the given half of block S: one
                # lo-table + one hi-table gather; the merging DVE add is
                # emitted later (emit_merge) once the DMA has surely landed,
                # so it never head-blocks the in-order DVE queue
                pn = q1 - q0
                mt = S["mt0"] if half == 0 else S["mt1"]
                ia, ib, s = S["ia"], S["ib"], S["slot"]
                nc.gpsimd.dma_gather(
                    out_ap=mt[:, 0:1, q0:q1], in_ap=xlo[:],
                    idxs_ap=ia[:, s, half, q0 // 16:q1 // 16],
                    num_idxs=pn, num_idxs_reg=pn, elem_size=F, transpose=True,
                    single_packet=False)
                sc = pmt.tile([128, 1, 2560], bf16, tag="sc", bufs=4, name="sc")
                nc.gpsimd.dma_gather(
                    out_ap=sc[:, :, 0:pn], in_ap=xhi[:],
                    idxs_ap=ib[:, s, half, q0 // 16:q1 // 16],
                    num_idxs=pn, num_idxs_reg=pn, elem_size=F, transpose=True,
                    single_packet=False)
                return (S, half, q0, q1, sc)

            def emit_merge(t, defer=True):
                # deferred merges get low priority: the list scheduler slots
                # them into DVE idle time once the gather DMA has landed,
                # instead of head-blocking the cell-update stream
                S, half, q0, q1, sc = t
                pn = q1 - q0
                mt = S["mt0"] if half == 0 else S["mt1"]
                with tc.high_priority(offset=-1000000 if defer else 0):
                    # <=1280-col pieces: a mis-slotted piece head-blocks the
                    # DVE queue for at most ~0.7us
                    npc = (pn + 1279) // 1280
                    step = (pn + npc - 1) // npc // 16 * 16
                    for p0 in range(0, pn, step):
                        p1 = min(pn, p0 + step)
                        nc.vector.tensor_add(out=mt[:, 0, q0 + p0:q0 + p1],
                                             in0=mt[:, 0, q0 + p0:q0 + p1],
                                             in1=sc[:, 0, p0:p1])

            def emit_gather_part(S, half, q0, q1, defer=True):
                emit_merge(emit_gather_pair(S, half, q0, q1), defer=defer)

            def alloc_mt(S):
                s = S["slot"]
                S["mt0"] = pmt.tile([128, 1, 4096], bf16, tag=f"mt0{s}", bufs=1,
                                    name=f"mt0{s}")
                S["mt1"] = pmt.tile([128, 1, 4608], bf16, tag=f"mt1{s}", bufs=1,
                                    name=f"mt1{s}")

            def emit_step_mm_sigma(S, d):
                w = S["w"]
                mt = S["mt0"] if d < 8 else S["mt1"]
                col = (d % 8) * w
                g = pps.tile([128, 2048], fp32, tag="g")
                S["g"] = g
                g0 = 1 if d == 0 else 0   # f-gate dead at d==0 (c0 = 0)
                # PSUM accumulation groups operate on 2KB zero regions: for
                # w=512 each gate bank owns a region (per-bank start/stop);
                # for w=128 all four banks share one region (single group).
                for gi in range(g0, 4):
                    nc.tensor.matmul(
                        out=g[:, gi * w:(gi + 1) * w],
                        lhsT=w_ih[:, gi * 128:(gi + 1) * 128],
                        rhs=mt[:, 0, col:col + w],
                        start=(w == 512 or gi == g0),
                        stop=(d == 0 and (w == 512 or gi == 3)))
                if d > 0:
                    for gi in range(4):
                        nc.tensor.matmul(
                            out=g[:, gi * w:(gi + 1) * w],
                            lhsT=w_hh[:, gi * 128:(gi + 1) * 128],
                            rhs=S["h"][:, 0:w], start=False,
                            stop=(w == 512 or gi == 3))
                sifo = pwk.tile([128, 2048], bf16, tag="sifo", bufs=4)
                if use_bias_g:
                    for k in range(g0, 4):
                        nc.scalar.activation(
                            out=sifo[:, k * w:(k + 1) * w],
                            in_=g[:, k * w:(k + 1) * w],
                            func=AF.Sigmoid, bias=bg_sb[:, k:k + 1])
                else:
                    nc.scalar.activation(out=sifo[:, g0 * w:4 * w],
                                         in_=g[:, g0 * w:4 * w], func=AF.Sigmoid)
                S["sifo"] = sifo

            def emit_step_rest(S, d):
                # DVE cell update + tanh(c) + h; emitted one slot late so the
                # in-order ACT queue never head-blocks on the DVE chain
                w, s, sifo = S["w"], S["slot"], S["sifo"]
                c = pst.tile([128, 512], bf16, tag=f"c{s}")
                if d == 0:
                    nc.vector.tensor_scalar(
                        out=c[:, 0:w], in0=sifo[:, 3 * w:4 * w], scalar1=2.0,
                        scalar2=1.0, op0=OP.mult, op1=OP.subtract)
                    nc.vector.tensor_mul(out=c[:, 0:w], in0=c[:, 0:w],
                                         in1=sifo[:, w:2 * w])
                else:
                    nc.vector.tensor_scalar(
                        out=sifo[:, 3 * w:4 * w], in0=sifo[:, 3 * w:4 * w],
                        scalar1=2.0, scalar2=1.0, op0=OP.mult, op1=OP.subtract)
                    c2 = pwk.tile([128, 512], bf16, tag="c2", bufs=3)
                    nc.vector.tensor_mul(out=c2[:, 0:w], in0=sifo[:, 0:w],
                                         in1=S["c"][:, 0:w])
                    nc.vector.tensor_mul(out=c[:, 0:w], in0=sifo[:, 3 * w:4 * w],
                                         in1=sifo[:, w:2 * w])
                    nc.vector.tensor_add(out=c[:, 0:w], in0=c[:, 0:w],
                                         in1=c2[:, 0:w])
                S["c"] = c
                tc_ = pwk.tile([128, 512], bf16, tag="tc", bufs=3)
                nc.scalar.activation(out=tc_[:, 0:w], in_=c[:, 0:w], func=AF.Tanh)
                h = pst.tile([128, 512], bf16, tag=f"h{s}")
                nc.vector.tensor_mul(out=h[:, 0:w], in0=sifo[:, 2 * w:3 * w],
                                     in1=tc_[:, 0:w])
                S["h"] = h

            def emit_final_psum(S):
                # rst directly in node-major: rst[node,f'] = xT.T @ W_selfT
                # + hT.T @ W_neighT; copied out of PSUM immediately so the
                # gates ring never backs up at a rotation boundary.
                w, ns = S["w"], S["w"] // 128
                xT = S["mt1"][:, 0, 8 * w:9 * w]
                rp = pps.tile([128, 2048], fp32, tag="g")
                v = rp[:, 0:512].rearrange("p (s f) -> p s f", s=4)
                for s4 in range(ns):
                    nc.tensor.matmul(
                        out=v[:, s4, :],
                        lhsT=xT[:, s4 * 128:(s4 + 1) * 128],
                        rhs=w_s[:], start=True, stop=False)
                    nc.tensor.matmul(
                        out=v[:, s4, :],
                        lhsT=S["h"][:, s4 * 128:(s4 + 1) * 128],
                        rhs=w_n[:], start=False, stop=True)
                rst = pfin.tile([128, 4, F], bf16, tag="rst", bufs=6)
                nc.vector.tensor_copy(out=rst[:, 0:ns], in_=v[:, 0:ns])
                if use_bias_o:
                    for s4 in range(ns):
                        nc.vector.tensor_add(out=rst[:, s4, :], in0=rst[:, s4, :],
                                             in1=bo_sb[:])
                S["rst"] = rst

            def ln_stats(S, key, srckey):
                ns = S["w"] // 128
                src = S[srckey]
                st = pfin.tile([128, 4, 6], fp32, tag="lnst", bufs=3, name="st")
                mv = pfin.tile([128, 4, 2], fp32, tag="lnmv", bufs=3, name="mv")
                for s in range(ns):
                    nc.vector.bn_stats(out=st[:, s, :], in_=src[:, s, :])
                    nc.vector.bn_aggr(out=mv[:, s, :], in_=st[:, s, :])
                S[key] = mv

            def ln_apply(S, dst, src, mv, aff, gsb, bsb):
                ns = S["w"] // 128
                ve = pfin.tile([128, 4], fp32, tag="lnve")
                nc.vector.tensor_scalar(
                    out=ve[:, 0:ns], in0=mv[:, 0:ns, 1], scalar1=EPS, scalar2=None,
                    op0=OP.add, op1=OP.bypass)
                rstd = pfin.tile([128, 4], fp32, tag="lnrstd")
                rsqrt_dve(rstd[:, 0:ns], ve[:, 0:ns], ns)
                for s in range(ns):
                    nc.vector.tensor_scalar(
                        out=dst[:, s, :], in0=src[:, s, :],
                        scalar1=mv[:, s, 0:1], scalar2=rstd[:, s:s + 1],
                        op0=OP.subtract, op1=OP.mult)
                    if aff:
                        nc.vector.tensor_mul(out=dst[:, s, :], in0=dst[:, s, :], in1=gsb[:])
                        nc.vector.tensor_add(out=dst[:, s, :], in0=dst[:, s, :], in1=bsb[:])
                dstf = dst[:, 0:ns].rearrange("p s f -> p (s f)")
                nc.vector.scalar_tensor_tensor(
                    out=dstf, in0=dstf, scalar=0.01, in1=dstf,
                    op0=OP.mult, op1=OP.max)

            def fin_ln1s(S):
                ln_stats(S, "mv1", "rst")

            def fin_ln1a(S):
                rst, ns = S["rst"], S["w"] // 128
                ln_apply(S, rst, rst, S["mv1"],
                         ln1_aff, g1_sb if ln1_aff else None,
                         b1_sb if ln1_aff else None)
                h2 = pfin.tile([128, 4, F], bf16, tag="h2", bufs=3)
                q = 4 * S["b"]
                nc.vector.tensor_add(out=h2[:, 0:ns], in0=rst[:, 0:ns],
                                     in1=xb_all[:, q:q + ns, :])
                S["h2"] = h2

            def fin_ln3s(S):
                ln_stats(S, "mv3", "h2")

            def fin_ln3a(S):
                b, ns = S["b"], S["w"] // 128
                ot = pfin.tile([128, 4, F], bf16, tag="ot", bufs=3)
                ln_apply(S, ot, S["h2"], S["mv3"],
                         ln3_aff, g3_sb if ln3_aff else None,
                         b3_sb if ln3_aff else None)
                nc.sync.dma_start(out=out_v[:, 4 * b:4 * b + ns, :], in_=ot[:, 0:ns])

            FIN_CHAIN = (fin_ln1s, fin_ln1a, fin_ln3s, fin_ln3a)

            # ---------------- schedule ----------------
            states = {}

            def init_states(rot, ia, ib):
                for s, b in enumerate(rot):
                    S = states[b] = {"b": b, "slot": s, "w": WIDTHS[b],
                                     "ia": ia, "ib": ib}
                    alloc_mt(S)
                return [states[b] for b in rot]

            pending = [None]
            pending_ln = []
            pending_fin = []
            inline_fin = [False]    # last rotation: finals inline (no successor)

            def finish_block(S):
                if inline_fin[0]:
                    emit_final_psum(S)
                    for fn in FIN_CHAIN:
                        fn(S)
                else:
                    pending_fin.append(S)

            def emit_mm_sigma_lagged(S, d):
                emit_step_mm_sigma(S, d)
                prev, pending[0] = pending[0], (S, d)
                if prev is not None:
                    emit_step_rest(prev[0], prev[1])
                    if prev[1] == D - 1:
                        finish_block(prev[0])

            def flush_pending():
                prev, pending[0] = pending[0], None
                if prev is not None:
                    emit_step_rest(prev[0], prev[1])
                    if prev[1] == D - 1:
                        finish_block(prev[0])

            def dribble_fin():
                # one unit of deferred final work per slot: first the PSUM
                # matmul+copy of a finished block, then its LN closures
                if pending_fin:
                    S = pending_fin.pop(0)
                    emit_final_psum(S)
                    pending_ln.extend((fn, S) for fn in FIN_CHAIN)
                elif pending_ln:
                    fn, S = pending_ln.pop(0)
                    fn(S)

            # prologue: rotation 0 half-A parts in need-time order (the
            # idx DMA was emitted first, ahead of the weight loads)
            rs = init_states(ROTS[0], ia0, ib0)
            parts = []
            for S in rs:
                w = S["w"]
                bnds = [0, w, 3 * w, 8 * w]
                for p in range(len(bnds) - 1):
                    key = (bnds[p] // w) * 5 + S["slot"]
                    parts.append((key, S["slot"], S, 0, bnds[p], bnds[p + 1]))
            parts.sort(key=lambda t: (t[0], t[1]))
            for _, _, S, half, q0, q1 in parts:
                emit_gather_part(S, half, q0, q1, defer=False)

            for r, rot in enumerate(ROTS):
                nxt = ROTS[r + 1] if r + 1 < len(ROTS) else None
                inline_fin[0] = nxt is None
                cur = [states[b] for b in rot]
                for d in range(1 if r > 0 else 0, D):
                    # gather parts to spread one-per-slot over this step;
                    # their merge-adds run 3 slots later (the DMA has landed
                    # by then, so the DVE queue never parks on a gather)
                    gq = []
                    if d in (0, 1) and r == 0:
                        for S in cur:
                            hb = 9 * S["w"]
                            mid = (hb // 2) // 128 * 128
                            q0, q1 = (0, mid) if d == 0 else (mid, hb)
                            gq.append((S, 1, q0, q1))
                    if d == 6 and nxt is not None:
                        with tc.high_priority(offset=-1000000):
                            nia, nib = load_rot_idx(nxt)
                        nxt_s = init_states(nxt, nia, nib)
                    if d in (7, 8) and nxt is not None:
                        for S2 in nxt_s:
                            ha = 8 * S2["w"]
                            q0, q1 = (0, ha // 2) if d == 7 else (ha // 2, ha)
                            gq.append((S2, 0, q0, q1))
                    if d in (10, 11) and nxt is not None:
                        for S2 in nxt_s:
                            hb = 9 * S2["w"]
                            mid = (hb // 2) // 128 * 128
                            q0, q1 = (0, mid) if d == 10 else (mid, hb)
                            gq.append((S2, 1, q0, q1))
                    for S in cur:
                        mark(f"r{r}d{d}s{S['slot']}")
                        emit_mm_sigma_lagged(S, d)
                        if gq:
                            emit_gather_part(*gq.pop(0))
                        if d >= 1:
                            dribble_fin()
                    for t in gq:
                        emit_gather_part(*t)
                    if d == 2 and r == 0:
                        with tc.high_priority(offset=-1000000):
                            nc.sync.dma_start(out=xb_all[:], in_=xsh_v[:])
                # boundary: next rotation's step 0 (no h dependency) goes
                # ahead of this rotation's lagged final work
                if nxt is not None:
                    for S2 in nxt_s:
                        mark(f"r{r+1}d0s{S2['slot']}-ovl")
                        emit_mm_sigma_lagged(S2, 0)
                flush_pending()
            while pending_fin or pending_ln:
                dribble_fin()

    nc.compile()
    return nc


def _wrap16(vals):
    # vals [..., n] -> [..., 128, n//16] int16 (16-wrap, x8 replicate)
    lead = vals.shape[:-1]
    n = vals.shape[-1]
    w = vals.reshape(*lead, n // 16, 16)
    w = np.moveaxis(w, -1, -2)                     # [..., 16, n/16]
    w = np.broadcast_to(w[..., None, :, :], (*lead, 8, 16, n // 16))
    return np.ascontiguousarray(w.reshape(*lead, 128, n // 16)).astype(np.int16)


def kernel(x, neigh_idx, W_self, b_self, W_neigh, b_neigh,
           W_ih, W_hh, b_ih, b_hh, g1, bt1, g3, bt3):
    bf = ml_dtypes.bfloat16
    x = np.asarray(x, np.float32)
    neigh_idx = np.asarray(neigh_idx, np.int32)

    xlo = np.zeros((NLO, F), bf)
    xlo[:SPLIT] = x[:SPLIT]                     # row 32767 stays zero
    xhi = np.zeros((NHI, F), bf)
    xhi[1:1 + (N - SPLIT)] = x[SPLIT:]          # row 0 stays zero

    # gate order in reference: i, f, g, o ; we use banks (f, i, o, g)
    perm = np.concatenate([np.arange(128, 256), np.arange(0, 128),
                           np.arange(384, 512), np.arange(256, 384)])
    gscale = np.ones((1, 512), np.float32)
    gscale[:, 384:512] = 2.0                    # tanh gate via 2*sigmoid(2g)-1
    W_ihT = np.ascontiguousarray(
        (np.asarray(W_ih, np.float32).T[:, perm] * gscale)).astype(bf)
    W_hhT = np.ascontiguousarray(
        (np.asarray(W_hh, np.float32).T[:, perm] * gscale)).astype(bf)
    bgv = (np.asarray(b_ih, np.float32) + np.asarray(b_hh, np.float32))[perm]
    bgv = bgv * gscale[0]
    bg2 = np.ascontiguousarray(bgv.reshape(4, F).T)                        # [F, 4]
    W_selfT = np.ascontiguousarray(np.asarray(W_self, np.float32).T).astype(bf)
    W_neighT = np.ascontiguousarray(np.asarray(W_neigh, np.float32).T).astype(bf)
    bov = np.ascontiguousarray(np.broadcast_to(
        np.asarray(b_self, np.float32) + np.asarray(b_neigh, np.float32), (128, F)))

    g1 = np.asarray(g1, np.float32); bt1 = np.asarray(bt1, np.float32)
    g3 = np.asarray(g3, np.float32); bt3 = np.asarray(bt3, np.float32)
    use_bias_g = bool(np.any(bgv != 0))
    use_bias_o = bool(np.any(bov != 0))
    ln1_aff = bool(np.any(g1 != 1) or np.any(bt1 != 0))
    ln3_aff = bool(np.any(g3 != 1) or np.any(bt3 != 0))
    g1t = np.ascontiguousarray(np.broadcast_to(g1, (128, F)))
    b1t = np.ascontiguousarray(np.broadcast_to(bt1, (128, F)))
    g3t = np.ascontiguousarray(np.broadcast_to(g3, (128, F)))
    b3t = np.ascontiguousarray(np.broadcast_to(bt3, (128, F)))

    key = (use_bias_g, use_bias_o, ln1_aff, ln3_aff)
    if key not in _CACHE:
        _CACHE[key] = _build(*key)
    nc = _CACHE[key]

    starts = np.cumsum([0] + WIDTHS)

    in_maps = []
    for core in range(NCORES):
        lo_r, hi_r = core * SHARD, (core + 1) * SHARD
        ni_pad = np.zeros((PAD, D), np.int64)
        ni_pad[:SHARD] = neigh_idx[lo_r:hi_r]
        xs_pad = np.zeros((PAD, F), bf)
        xs_pad[:SHARD] = x[lo_r:hi_r]
        # own global row id per padded node (pad rows -> lo zero row)
        own = np.full(PAD, SPLIT, np.int64)
        own[:SHARD] = np.arange(lo_r, hi_r)

        ia_list = np.full((NBLK, 2, 16 * IDXC), SPLIT, np.int64)
        ib_list = np.zeros((NBLK, 2, 16 * IDXC), np.int64)
        for b in range(NBLK):
            w, ha, hb = _geom(b)
            j = np.arange(D * w)
            node = j % w
            dd = j // w
            rows = ni_pad[starts[b] + node, dd]          # [D*w]
            rows = np.concatenate([rows, own[starts[b]:starts[b] + w]])
            lo_mask = rows < SPLIT
            ia = np.where(lo_mask, rows, SPLIT)
            ib = np.where(lo_mask, 0, rows - SPLIT + 1)
            ia_list[b, 0, :ha] = ia[:ha]
            ia_list[b, 1, :hb] = ia[ha:]
            ib_list[b, 0, :ha] = ib[:ha]
            ib_list[b, 1, :hb] = ib[ha:]

        idxt = np.concatenate([_wrap16(ia_list), _wrap16(ib_list)], axis=-1)
        in_maps.append(dict(
            xlo=xlo, xhi=xhi, xsh=xs_pad, idxt=idxt,
            wih=W_ihT, whh=W_hhT, wself=W_selfT, wneigh=W_neighT,
            bg=bg2, bo=bov, g1t=g1t, b1t=b1t, g3t=g3t, b3t=b3t,
        ))

    res = run_bass_kernel_spmd(nc, in_maps, core_ids=list(range(NCORES)))
    kernel.last_results = res
    out = np.concatenate([res.results[c]["out"][:SHARD] for c in range(NCORES)], 0)
    return out.astype(np.float32)
